# revision 1
# baseline (speedup 1.0000x reference)
"""Trainium2 Bass kernel for the DGNN_GA heterograph SAGE + edge-head model.

Strategy (self-contained; shapes derived from inputs at runtime):
- Host shards edges two ways: by t//S (shard-T) and by a//S (shard-A), S = NT/8.
  Within each core's shard, edges are sorted by the owning node and padded so
  every 128-node window occupies a fixed number of 128-edge chunks (CPW),
  identical across cores (SPMD requires one program).
- Device (per core): encoders -> node tables [N+1, 65] (col64 = count flag);
  sweep over shard-A edges: [128,1]-indirect gather of ht rows + one-hot
  segment-sum matmul into PSUM -> m_a (own slice); fold weights -> u slice;
  AllGather u. Same over shard-T -> m_t -> v slice (stays local). Final sweep:
  gather u[a_e], one-hot expand v[t_e], s = relu(u+v), logits = s.w2 + b2.
- Host unpads/unsorts logits back to the original edge order.
"""
LAST_EXEC_S = None
import time

import numpy as np

import concourse.bacc as bacc
import concourse.bass as bass
import concourse.mybir as mybir
import concourse.tile as tile
from concourse import bass_utils
from concourse.masks import make_identity

F32 = mybir.dt.float32
I32 = mybir.dt.int32
AX = mybir.AxisListType
OP = mybir.AluOpType
AF = mybir.ActivationFunctionType

N_CORES = 8
WIN = 128          # nodes per segment window
CH = 128           # edges per chunk
EPS = 1e-5


# ---------------------------------------------------------------- host prep

def _prep_shard(own, other, S, core):
    """Edges whose `own` node is in this core's slice, sorted by own-node,
    padded per 128-node window to whole chunks.
    Returns (own_local_sorted, other_sorted, orig_pos, per-window counts)."""
    lo = core * S
    sel = np.nonzero((own >= lo) & (own < lo + S))[0]
    ow = own[sel] - lo
    order = np.argsort(ow, kind="stable")
    return ow[order], other[sel][order], sel[order]


def _pack_windows(ow, other, pos, S, cpw, n_pad_node):
    """Lay out sorted edges into a fixed grid of n_win windows x cpw chunks.
    Pad slots: other-idx = n_pad_node (zero row), local = -1000 (no one-hot hit).
    Returns (other_idx [n], local_rel [n], orig [n]) with n = n_win*cpw*128."""
    n_win = (S + WIN - 1) // WIN
    n = n_win * cpw * CH
    oth = np.full(n, n_pad_node, np.int32)
    rel = np.full(n, -1000.0, np.float32)
    org = np.full(n, -1, np.int64)
    wid = ow // WIN
    starts = np.searchsorted(wid, np.arange(n_win + 1))
    for w in range(n_win):
        s, e = starts[w], starts[w + 1]
        k = e - s
        assert k <= cpw * CH
        base = w * cpw * CH
        oth[base:base + k] = other[s:e]
        rel[base:base + k] = (ow[s:e] - w * WIN).astype(np.float32)
        org[base:base + k] = pos[s:e]
    return oth, rel, org


def _wrap128(x):
    """[n] -> [128, n/128] with edge i at [i%128, i//128]."""
    return np.ascontiguousarray(x.reshape(-1, CH).T)


def _row16(x):
    """[n] -> [16, ceil(nch/16)*128]: chunk j's 128 values as a row segment at
    partition j%16, cols (j//16)*128."""
    nch = x.size // CH
    g = (nch + 15) // 16
    out = np.zeros((16, g * CH), x.dtype)
    for j in range(nch):
        out[j % 16, (j // 16) * CH:(j // 16 + 1) * CH] = x[j * CH:(j + 1) * CH]
    return out


# ---------------------------------------------------------------- device build

def _encoder(nc, tc, pool, ppool, x_dram, W_t, b_rep, g_rep, bg_rep, out_tab,
             N, D, ident):
    """relu(LN(x @ W.T + b)) -> out_tab rows [N, 0:64], col64 = 1.0."""
    H = 64
    for r0 in range(0, N, 128):
        cnt = min(128, N - r0)
        xt = pool.tile([128, D], F32, name=f"xt_{out_tab.name}_{r0}", tag="enc_x")
        nc.sync.dma_start(xt[:cnt], x_dram[r0:r0 + cnt, :])
        xT_p = ppool.tile([D, 128], F32, name=f"xTp_{out_tab.name}_{r0}",
                          tag="tpa", bufs=1, space="PSUM")
        nc.tensor.transpose(xT_p[:, :cnt], xt[:cnt], ident[:cnt, :cnt])
        xT = pool.tile([D, 128], F32, name=f"xT_{out_tab.name}_{r0}", tag="enc_xT")
        nc.vector.tensor_copy(xT[:, :cnt], xT_p[:, :cnt])
        h_p = ppool.tile([128, H], F32, name=f"hp_{out_tab.name}_{r0}",
                         tag="tpb", bufs=2, space="PSUM")
        nc.tensor.matmul(h_p[:cnt], xT[:, :cnt], W_t[:], start=True, stop=True)
        o = pool.tile([128, 65], F32, name=f"o_{out_tab.name}_{r0}", tag="enc_o")
        h = o[:cnt, 0:64]
        nc.vector.tensor_tensor(out=h, in0=h_p[:cnt], in1=b_rep[:cnt], op=OP.add)
        mu = pool.tile([128, 1], F32, name=f"mu_{out_tab.name}_{r0}", tag="enc_mu")
        nc.vector.tensor_reduce(out=mu[:cnt], in_=h, axis=AX.X, op=OP.add)
        # d = h - mu/64
        nc.vector.scalar_tensor_tensor(out=h, in0=mu[:cnt].to_broadcast([cnt, H]),
                                       scalar=-1.0 / H, in1=h, op0=OP.mult, op1=OP.add)
        sq = pool.tile([128, H], F32, name=f"sq_{out_tab.name}_{r0}", tag="enc_sq")
        nc.vector.tensor_mul(sq[:cnt], h, h)
        var = pool.tile([128, 1], F32, name=f"var_{out_tab.name}_{r0}", tag="enc_var")
        nc.vector.tensor_reduce(out=var[:cnt], in_=sq[:cnt], axis=AX.X, op=OP.add)
        rstd = pool.tile([128, 1], F32, name=f"rst_{out_tab.name}_{r0}", tag="enc_rst")
        nc.vector.tensor_scalar(out=var[:cnt], in0=var[:cnt], scalar1=1.0 / H,
                                scalar2=EPS, op0=OP.mult, op1=OP.add)
        nc.scalar.activation(rstd[:cnt], var[:cnt], AF.Sqrt)
        nc.vector.reciprocal(rstd[:cnt], rstd[:cnt])
        nc.vector.tensor_mul(h, h, rstd[:cnt].to_broadcast([cnt, H]))
        nc.vector.tensor_mul(h, h, g_rep[:cnt])
        nc.vector.tensor_tensor(out=h, in0=h, in1=bg_rep[:cnt], op=OP.add)
        nc.vector.tensor_relu(h, h)
        nc.vector.memset(o[:cnt, 64:65], 1.0)
        nc.sync.dma_start(out_tab[r0:r0 + cnt, :], o[:cnt])


def _sweep(nc, tc, pool, ppool, tag, src_tab, idx_sb, rel_sb, iota_full, mT, n_win,
           cpw):
    """Gather src rows by idx, segment-sum into mT [65, n_win*128] via one-hot."""
    for j in range(n_win * cpw):
        w, k = divmod(j, cpw)
        msgs = pool.tile([128, 65], F32, name=f"msg_{tag}_{j}", tag=f"msg_{tag}",
                         bufs=4)
        nc.gpsimd.indirect_dma_start(
            out=msgs[:], out_offset=None, in_=src_tab[:],
            in_offset=bass.IndirectOffsetOnAxis(ap=idx_sb[:, j:j + 1], axis=0))
        oh = pool.tile([128, WIN], F32, name=f"oh_{tag}_{j}", tag=f"oh_{tag}", bufs=4)
        nc.vector.tensor_tensor(
            out=oh[:], in0=rel_sb[:, j:j + 1].to_broadcast([128, WIN]),
            in1=iota_full[:], op=OP.is_equal)
        if k == 0:
            psum = ppool.tile([65, WIN], F32, name=f"ps_{tag}_{w}", tag="ps",
                              space="PSUM", bufs=2)
            _sweep.psum = psum
        psum = _sweep.psum
        nc.tensor.matmul(psum[:], msgs[:], oh[:], start=(k == 0), stop=(k == cpw - 1))
        if k == cpw - 1:
            nc.vector.tensor_copy(mT[:, w * WIN:(w + 1) * WIN], psum[:])


def _node_transform(nc, tc, pool, ppool, tag, mT, h_tab, hx_idx_sb, S, FA_t, FB_t,
                    c_rep, ones_row, ident, out_dram, out_sb):
    """out rows = (m/cnt) @ FA^T + h_slice @ FB^T + c  per 128-node window.
    h_slice rows fetched by indirect gather with per-core index data (SPMD)."""
    n_win = (S + WIN - 1) // WIN
    for w in range(n_win):
        cnt = min(WIN, S - w * WIN)
        mw = mT[:, w * WIN:w * WIN + cnt]
        rec = pool.tile([1, WIN], F32, name=f"rec_{tag}_{w}", tag=f"rec_{tag}")
        nc.vector.tensor_scalar_max(rec[:, :cnt], mw[64:65, :], 1.0)
        nc.vector.reciprocal(rec[:, :cnt], rec[:, :cnt])
        rec_p = ppool.tile([64, WIN], F32, name=f"recp_{tag}_{w}", tag="tpa", bufs=1,
                           space="PSUM")
        nc.tensor.matmul(rec_p[:, :cnt], ones_row[:, 0:64], rec[:, :cnt],
                         start=True, stop=True)
        mn = pool.tile([64, WIN], F32, name=f"mn_{tag}_{w}", tag=f"mn_{tag}")
        nc.vector.tensor_mul(mn[:, :cnt], mw[0:64, :], rec_p[:, :cnt])
        hx = pool.tile([128, 65], F32, name=f"hx_{tag}_{w}", tag=f"hx_{tag}")
        nc.gpsimd.indirect_dma_start(
            out=hx[:], out_offset=None, in_=h_tab[:],
            in_offset=bass.IndirectOffsetOnAxis(ap=hx_idx_sb[:, w:w + 1], axis=0))
        hxT_p = ppool.tile([65, 128], F32, name=f"hxTp_{tag}_{w}", tag="tpa", bufs=1,
                           space="PSUM")
        nc.tensor.transpose(hxT_p[:], hx[:], ident[:])
        hxT = pool.tile([65, 128], F32, name=f"hxT_{tag}_{w}", tag=f"hxT_{tag}")
        nc.vector.tensor_copy(hxT[:, :cnt], hxT_p[:, :cnt])
        u_p = ppool.tile([WIN, 64], F32, name=f"up_{tag}_{w}", tag="tpb", bufs=2,
                         space="PSUM")
        nc.tensor.matmul(u_p[:cnt], mn[:, :cnt], FA_t[:], start=True, stop=False)
        nc.tensor.matmul(u_p[:cnt], hxT[0:64, :cnt], FB_t[:], start=False, stop=True)
        usb = pool.tile([WIN, 64], F32, name=f"usb_{tag}_{w}", tag=f"usb_{tag}")
        nc.vector.tensor_tensor(out=usb[:cnt], in0=u_p[:cnt],
                                in1=c_rep[:cnt], op=OP.add)
        if out_dram is not None:
            nc.sync.dma_start(out_dram[w * WIN:w * WIN + cnt, :], usb[:cnt])
        if out_sb is not None:
            nc.vector.tensor_copy(out_sb[0:cnt, w * 64:(w + 1) * 64], usb[:cnt])


def _mat_T(nc, ppool, pool, src_sb, n, m, ident, name):
    """PE transpose [n, m] -> SBUF [m, n]."""
    p = ppool.tile([m, n], F32, name=name + "_p", tag="tpa", bufs=1, space="PSUM")
    nc.tensor.transpose(p[:], src_sb[:n, :m], ident[:n, :n])
    s = pool.tile([m, n], F32, name=name)
    nc.vector.tensor_copy(s[:], p[:])
    return s


def build_program(NA, NT, DA, DT, cpwA, cpwT, nchA, nchT):
    H = 64
    S_A, S_T = NA // N_CORES, NT // N_CORES
    nwA = (S_A + WIN - 1) // WIN
    nwT = (S_T + WIN - 1) // WIN

    nc = bacc.Bacc("TRN2", target_bir_lowering=False, debug=False,
                   num_devices=N_CORES)
    dt = nc.dram_tensor
    x_agent = dt("x_agent", [NA, DA], F32, kind="ExternalInput")
    x_task = dt("x_task", [NT, DT], F32, kind="ExternalInput")
    wnames = ["Wa", "ba", "ga", "bga", "Wt", "bt", "gt", "bgt",
              "Wl_at", "bl_at", "Wr_at", "Wl_ta", "bl_ta", "Wr_ta",
              "W1", "b1", "W2", "b2"]
    wshapes = {"Wa": [H, DA], "Wt": [H, DT], "W1": [H, 2 * H], "W2": [1, H],
               "Wl_at": [H, H], "Wr_at": [H, H], "Wl_ta": [H, H], "Wr_ta": [H, H],
               "b2": [1]}
    W = {n: dt(n, wshapes.get(n, [H]), F32, kind="ExternalInput") for n in wnames}
    tA_idx = dt("tA_idx", [128, nchA], I32, kind="ExternalInput")
    alA = dt("alA", [128, nchA], F32, kind="ExternalInput")
    aT_idx = dt("aT_idx", [128, nchT], I32, kind="ExternalInput")
    tlT = dt("tlT", [128, nchT], F32, kind="ExternalInput")
    hxA_idx = dt("hxA_idx", [128, nwA], I32, kind="ExternalInput")
    hxT_idx = dt("hxT_idx", [128, nwT], I32, kind="ExternalInput")
    logits_out = dt("logits_out", [128, nchT], F32, kind="ExternalOutput")

    ha_tab = dt("ha_tab", [NA + 1, 65], F32)
    ht_tab = dt("ht_tab", [NT + 1, 65], F32)
    u_slice = dt("u_slice", [S_A, 64], F32)
    u_full = dt("u_full", [NA + 1, 64], F32, addr_space="Shared")

    with tile.TileContext(nc) as tc:
        with (tc.tile_pool(name="p", bufs=2) as pool,
              tc.tile_pool(name="pp", bufs=2, space="PSUM") as ppool,
              tc.tile_pool(name="pc", bufs=1) as cpool):
            ident = cpool.tile([128, 128], F32, name="ident")
            make_identity(nc, ident[:])
            iota_i = cpool.tile([128, WIN], I32, name="iota_i")
            nc.gpsimd.iota(iota_i[:], pattern=[[1, WIN]], base=0, channel_multiplier=0)
            iota_full = cpool.tile([128, WIN], F32, name="iota_full")
            nc.vector.tensor_copy(iota_full[:], iota_i[:])
            zrow = cpool.tile([1, 65], F32, name="zrow")
            nc.vector.memset(zrow[:], 0.0)
            ones_row = cpool.tile([1, 128], F32, name="ones_row")
            nc.vector.memset(ones_row[:], 1.0)

            def prep_rep(row_ap, width, name):
                p = ppool.tile([128, width], F32, name=name + "_p", tag="tpa",
                               bufs=1, space="PSUM")
                nc.tensor.matmul(p[:], ones_row[:], row_ap, start=True, stop=True)
                t = cpool.tile([128, width], F32, name=name)
                nc.vector.tensor_copy(t[:], p[:])
                return t

            # ---- load weights to SBUF + transposes/folds
            wsb = {}
            for n in wnames:
                sh = wshapes.get(n, [H])
                if len(sh) == 1:
                    t = cpool.tile([1, sh[0]], F32, name=f"w_{n}")
                    nc.sync.dma_start(t[:], W[n][None, :])
                else:
                    t = cpool.tile(sh, F32, name=f"w_{n}")
                    nc.sync.dma_start(t[:], W[n][:])
                wsb[n] = t
            WaT = _mat_T(nc, ppool, pool, wsb["Wa"], H, DA, ident, "WaT")
            WtT = _mat_T(nc, ppool, pool, wsb["Wt"], H, DT, ident, "WtT")
            W1L = wsb["W1"][:, 0:H]
            W1R = wsb["W1"][:, H:2 * H]
            W1LT_p = ppool.tile([H, H], F32, name="W1LT_p", tag="tpa", bufs=1, space="PSUM")
            nc.tensor.transpose(W1LT_p[:], W1L, ident[:H, :H])
            W1LT = pool.tile([H, H], F32, name="W1LT")
            nc.vector.tensor_copy(W1LT[:], W1LT_p[:])
            W1RT_p = ppool.tile([H, H], F32, name="W1RT_p", tag="tpa", bufs=1, space="PSUM")
            nc.tensor.transpose(W1RT_p[:], W1R, ident[:H, :H])
            W1RT = pool.tile([H, H], F32, name="W1RT")
            nc.vector.tensor_copy(W1RT[:], W1RT_p[:])

            def fold(name, Wl, rhsT):
                p = ppool.tile([H, H], F32, name=name + "_p", tag="tpa", bufs=1, space="PSUM")
                nc.tensor.matmul(p[:], wsb[Wl][:], rhsT[:], start=True, stop=True)
                s = pool.tile([H, H], F32, name=name)
                nc.vector.tensor_copy(s[:], p[:])
                return s
            FA_t = fold("FA_t", "Wl_ta", W1LT)   # (W1L @ Wl_ta)^T
            FB_t = fold("FB_t", "Wr_ta", W1LT)
            FC_t = fold("FC_t", "Wl_at", W1RT)
            FD_t = fold("FD_t", "Wr_at", W1RT)
            # cu = W1L @ bl_ta + b1 ; cv = W1R @ bl_at
            def foldb(name, bl, rhsT, extra):
                blc = cpool.tile([H, 1], F32, name=name + "_c")
                nc.sync.dma_start(blc[:], W[bl][:, None])   # [H] -> [H,1] column
                p = ppool.tile([1, H], F32, name=name + "_p", tag="tpa", bufs=1, space="PSUM")
                nc.tensor.matmul(p[:], blc[:], rhsT[:], start=True, stop=True)
                s = pool.tile([1, H], F32, name=name)
                if extra is not None:
                    nc.vector.tensor_tensor(out=s[:], in0=p[:], in1=extra[:],
                                            op=OP.add)
                else:
                    nc.vector.tensor_copy(s[:], p[:])
                return s
            cu = foldb("cu", "bl_ta", W1LT, wsb["b1"])
            cv = foldb("cv", "bl_at", W1RT, None)
            cu_rep = prep_rep(cu[:], 64, "cu_rep")
            cv_rep = prep_rep(cv[:], 64, "cv_rep")
            ba_rep = prep_rep(wsb["ba"][:], 64, "ba_rep")
            ga_rep = prep_rep(wsb["ga"][:], 64, "ga_rep")
            bga_rep = prep_rep(wsb["bga"][:], 64, "bga_rep")
            bt_rep = prep_rep(wsb["bt"][:], 64, "bt_rep")
            gt_rep = prep_rep(wsb["gt"][:], 64, "gt_rep")
            bgt_rep = prep_rep(wsb["bgt"][:], 64, "bgt_rep")
            w2_rep = prep_rep(wsb["W2"][:], 64, "w2_rep")
            b2_rep = prep_rep(wsb["b2"][:], 1, "b2_rep")

            # ---- encoders
            _encoder(nc, tc, pool, ppool, x_agent, WaT, ba_rep, ga_rep,
                     bga_rep, ha_tab, NA, DA, ident)
            _encoder(nc, tc, pool, ppool, x_task, WtT, bt_rep, gt_rep,
                     bgt_rep, ht_tab, NT, DT, ident)
            nc.sync.dma_start(ha_tab[NA:NA + 1, :], zrow[:])
            nc.sync.dma_start(ht_tab[NT:NT + 1, :], zrow[:])

            # ---- load edge metadata
            tA_sb = cpool.tile([128, nchA], I32, name="tA_sb")
            nc.sync.dma_start(tA_sb[:], tA_idx[:])
            alA_sb = cpool.tile([128, nchA], F32, name="alA_sb")
            nc.sync.dma_start(alA_sb[:], alA[:])
            aT_sb = cpool.tile([128, nchT], I32, name="aT_sb")
            nc.sync.dma_start(aT_sb[:], aT_idx[:])
            tlT_sb = cpool.tile([128, nchT], F32, name="tlT_sb")
            nc.sync.dma_start(tlT_sb[:], tlT[:])
            hxA_sb = cpool.tile([128, nwA], I32, name="hxA_sb")
            nc.sync.dma_start(hxA_sb[:], hxA_idx[:])
            hxT_sb = cpool.tile([128, nwT], I32, name="hxT_sb")
            nc.sync.dma_start(hxT_sb[:], hxT_idx[:])

            # ---- sweep A: m_a, u slice, allgather
            maT = cpool.tile([65, nwA * WIN], F32, name="maT")
            _sweep(nc, tc, pool, ppool, "A", ht_tab, tA_sb, alA_sb, iota_full,
                   maT[:], nwA, cpwA)
            _node_transform(nc, tc, pool, ppool, "u", maT[:], ha_tab, hxA_sb, S_A,
                            FA_t, FB_t, cu_rep, ones_row, ident, u_slice, None)
            nc.gpsimd.collective_compute(
                "AllGather", OP.bypass, replica_groups=[list(range(N_CORES))],
                ins=[u_slice[:]], outs=[u_full[0:NA, :]])
            nc.sync.dma_start(u_full[NA:NA + 1, :], zrow[:, 0:64])

            # ---- sweep T: m_t, v slice (local)
            mtT = cpool.tile([65, nwT * WIN], F32, name="mtT")
            _sweep(nc, tc, pool, ppool, "T", ha_tab, aT_sb, tlT_sb, iota_full,
                   mtT[:], nwT, cpwT)
            v_sb = cpool.tile([128, nwT * 64], F32, name="v_sb")
            nc.vector.memset(v_sb[:], 0.0)
            _node_transform(nc, tc, pool, ppool, "v", mtT[:], ht_tab, hxT_sb, S_T,
                            FC_t, FD_t, cv_rep, ones_row, ident, None, v_sb)

            # ---- final edge sweep (shard-T): logits
            logits_sb = cpool.tile([128, nchT], F32, name="logits_sb")
            for j in range(nchT):
                w = j // cpwT
                ug = pool.tile([128, 64], F32, name=f"ug_{j}", tag="ug", bufs=4)
                nc.gpsimd.indirect_dma_start(
                    out=ug[:], out_offset=None, in_=u_full[:],
                    in_offset=bass.IndirectOffsetOnAxis(ap=aT_sb[:, j:j + 1], axis=0))
                oh = pool.tile([128, CH], F32, name=f"oh4_{j}", tag="oh4", bufs=4)
                nc.vector.tensor_tensor(
                    out=oh[:], in0=tlT_sb[:, j:j + 1].to_broadcast([128, CH]),
                    in1=iota_full[:], op=OP.is_equal)
                oh2_p = ppool.tile([CH, 128], F32, name=f"oh2p_{j}", tag="tpa",
                                   bufs=1, space="PSUM")
                nc.tensor.transpose(oh2_p[:], oh[:], ident[:])
                oh2 = pool.tile([CH, 128], F32, name=f"oh2_{j}", tag="oh2", bufs=4)
                nc.vector.tensor_copy(oh2[:], oh2_p[:])
                v_p = ppool.tile([CH, 64], F32, name=f"vp_{j}", tag="vp", bufs=2,
                                 space="PSUM")
                nc.tensor.matmul(v_p[:], oh2[:], v_sb[:, w * 64:(w + 1) * 64],
                                 start=True, stop=True)
                s = pool.tile([128, 64], F32, name=f"s_{j}", tag="s", bufs=4)
                nc.vector.tensor_add(s[:], ug[:], v_p[:])
                nc.vector.tensor_relu(s[:], s[:])
                nc.vector.tensor_mul(s[:], s[:], w2_rep[:])
                lg = pool.tile([128, 1], F32, name=f"lg_{j}", tag="lg", bufs=4)
                nc.vector.tensor_reduce(out=lg[:], in_=s[:], axis=AX.X, op=OP.add)
                nc.vector.tensor_tensor(out=logits_sb[:, j:j + 1], in0=lg[:],
                                        in1=b2_rep[:], op=OP.add)
            nc.sync.dma_start(logits_out[:], logits_sb[:])
    nc.compile()
    return nc


# ---------------------------------------------------------------- entry point

def kernel(**inputs):
    inputs = {k: np.asarray(v) for k, v in inputs.items()}
    a = inputs["edge_index"][0].astype(np.int32)
    t = inputs["edge_index"][1].astype(np.int32)
    NA, DA = inputs["x_agent"].shape
    NT, DT = inputs["x_task"].shape
    E = a.size
    S_A, S_T = NA // N_CORES, NT // N_CORES
    nwA = (S_A + WIN - 1) // WIN
    nwT = (S_T + WIN - 1) // WIN

    shardA = [_prep_shard(a, t, S_A, c) for c in range(N_CORES)]
    shardT = [_prep_shard(t, a, S_T, c) for c in range(N_CORES)]

    def cpw_of(shards, n_win):
        m = 1
        for ow, _, _ in shards:
            wid = ow // WIN
            cnt = np.bincount(wid, minlength=n_win).max() if ow.size else 1
            m = max(m, -(-int(cnt) // CH))
        return m
    cpwA = cpw_of(shardA, nwA)
    cpwT = cpw_of(shardT, nwT)
    nchA, nchT = nwA * cpwA, nwT * cpwT

    per_core = []
    origs = []
    for c in range(N_CORES):
        owA, othA, _ = shardA[c]
        tA, alA_, _ = _pack_windows(owA, othA, np.zeros_like(owA), S_A, cpwA, NT)
        owT, othT, posT = shardT[c]
        aT, tlT_, orgT = _pack_windows(owT, othT, posT, S_T, cpwT, NA)
        origs.append(orgT)
        m = {k: np.ascontiguousarray(v) for k, v in inputs.items()
             if k not in ("edge_index",)}
        m["tA_idx"] = _wrap128(tA)
        m["alA"] = _wrap128(alA_)
        m["aT_idx"] = _wrap128(aT)
        m["tlT"] = _wrap128(tlT_)

        def hx_idx(S, base, nzero, n_win):
            o = np.full((128, n_win), nzero, np.int32)
            for w in range(n_win):
                cnt = min(WIN, S - w * WIN)
                o[:cnt, w] = base + w * WIN + np.arange(cnt)
            return o
        m["hxA_idx"] = hx_idx(S_A, c * S_A, NA, nwA)
        m["hxT_idx"] = hx_idx(S_T, c * S_T, NT, nwT)
        per_core.append(m)

    nc = build_program(NA, NT, DA, DT, cpwA, cpwT, nchA, nchT)
    global LAST_EXEC_S
    t0 = time.perf_counter()
    res = bass_utils.run_bass_kernel_spmd(nc, per_core, list(range(N_CORES)))
    LAST_EXEC_S = time.perf_counter() - t0

    logits = np.zeros(E, np.float32)
    for c in range(N_CORES):
        out = res.results[c]["logits_out"]     # [128, nchT]
        vals = out.T.reshape(-1)               # edge i at [i%128, i//128]
        org = origs[c]
        ok = org >= 0
        logits[org[ok]] = vals[ok]
    return logits

